# revision 25
# baseline (speedup 1.0000x reference)
"""Trainium2 Bass kernel for nn_AttentionBlock (GroupNorm32 + 16-head self
attention + out-proj + residual), data-parallel over batch across 8 cores.

Contract: kernel(**inputs) takes the FULL unsharded inputs (as produced by
reference.setup_inputs) and returns the FULL [8, 1024, 1024] fp32 output.

Per-core algorithm (one batch element per NeuronCore, no collectives):
  A. GroupNorm in fp32. Per 128-channel tile: per-partition sum (DVE reduce)
     and sum-of-squares (ACT Square with accum_out), group reduction via a
     tiny block-diagonal matmul on PE, then xn = x*A - negB per partition.
  B. QKV as two matmuls in bf16. q/k produced channel-major (q rows pre-scaled
     by 1/64 on the host, biases folded in via K=1 ones-row matmuls);
     v produced transposed (positions-major) so attention needs no transposes.
  C. Per head: scoreT = k.T @ q (keys on PSUM partitions), exp on ACT into
     bf16 expT, then ctxT = [vT | ones].T @ expT -- the appended ones column
     yields the softmax denominator row for free.
  D. Normalize ctx by broadcast reciprocal denominators, out-proj matmul in
     bf16 (bias folded), fp32 residual with the GroupNorm output, DMA out.
"""

import sys
import numpy as np

if "/opt/trn_rl_repo" not in sys.path:
    sys.path.insert(0, "/opt/trn_rl_repo")

import ml_dtypes

B, C, L = 8, 1024, 1024
H, D = 16, 64
NT = 8            # 128-row tiles per 1024 dim
EPS = 1e-5
NELEM = 32 * L    # elements per GroupNorm group (32 channels x 1024)

_STATE = {}


def _build_nc(debug=False):
    import concourse.bacc as bacc
    import concourse.mybir as mybir
    import concourse.tile as tile

    f32 = mybir.dt.float32
    bf16 = mybir.dt.bfloat16
    AX = mybir.AxisListType
    ALU = mybir.AluOpType
    AF = mybir.ActivationFunctionType

    nc = bacc.Bacc("TRN2", target_bir_lowering=False, debug=False, num_devices=8)

    x_d = nc.dram_tensor("x", [C, L], f32, kind="ExternalInput").ap()
    gnw_d = nc.dram_tensor("gnw", [C], f32, kind="ExternalInput").ap()
    gnb_d = nc.dram_tensor("gnb", [C], f32, kind="ExternalInput").ap()
    wqk_d = nc.dram_tensor("wqk", [C, 2 * C], bf16, kind="ExternalInput").ap()
    bqk_d = nc.dram_tensor("bqk", [2 * C], f32, kind="ExternalInput").ap()
    wv_d = nc.dram_tensor("wv", [C, C], bf16, kind="ExternalInput").ap()
    bv_d = nc.dram_tensor("bv", [C], bf16, kind="ExternalInput").ap()
    wout_d = nc.dram_tensor("wout", [C, C], bf16, kind="ExternalInput").ap()
    bout_d = nc.dram_tensor("bout", [C], f32, kind="ExternalInput").ap()
    out_d = nc.dram_tensor("out", [C, L], f32, kind="ExternalOutput").ap()
    dbg = {}
    if debug:
        for nm, shp, dt in [
            ("dbg_stats", [128, 16], f32), ("dbg_xnf", [NT, 128, L], f32),
            ("dbg_qk", [H, 128, L], bf16), ("dbg_vt", [NT, 128, 16 * 65], bf16),
            ("dbg_exp0", [NT, 128, L], bf16), ("dbg_denom", [16, L], bf16),
            ("dbg_recip", [16, L], bf16), ("dbg_ctxu", [NT, 128, L], bf16),
            ("dbg_ctxn", [NT, 128, L], bf16),
        ]:
            dbg[nm] = nc.dram_tensor(nm, shp, dt, kind="ExternalOutput").ap()

    with tile.TileContext(nc) as tc:
        # ---------------- persistent pools ----------------
        from contextlib import ExitStack

        with ExitStack() as es:
            const = es.enter_context(tc.tile_pool(name="const", bufs=1))
            small = es.enter_context(tc.tile_pool(name="small", bufs=3))
            xnf_p = es.enter_context(tc.tile_pool(name="xnf", bufs=1))
            ctx_p = es.enter_context(tc.tile_pool(name="ctxp", bufs=1))
            qkv_p = es.enter_context(tc.tile_pool(name="qkvp", bufs=1))

            # constants and small parameters
            E = const.tile([128, 128], f32, tag="E", name="E")
            nc.gpsimd.memset(E[:], 0.0)
            for j in range(4):
                nc.gpsimd.memset(E[32 * j:32 * (j + 1), 32 * j:32 * (j + 1)], 1.0)
            ones_bf = const.tile([1, 512], bf16, tag="ones", name="ones")
            nc.gpsimd.memset(ones_bf[:], 1.0)
            # ones rows at partition 0 and 64 (broadcast matmuls need the
            # stationary ones row at the same base partition as the source)
            ones2 = const.tile([65, 64], bf16, tag="ones2", name="ones2")
            nc.gpsimd.memset(ones2[0:1, :], 1.0)
            nc.gpsimd.memset(ones2[64:65, :], 1.0)
            eps_t = const.tile([128, 1], f32, tag="eps", name="eps")
            nc.gpsimd.memset(eps_t[:], EPS)

            gnw_sb = const.tile([128, NT], f32, tag="gnw", name="gnw")
            nc.sync.dma_start(gnw_sb[:], gnw_d.rearrange("(t p) -> p t", p=128))
            gnb_sb = const.tile([128, NT], f32, tag="gnb", name="gnb")
            nc.sync.dma_start(gnb_sb[:], gnb_d.rearrange("(t p) -> p t", p=128))
            bqk_col = const.tile([128, H], f32, tag="bqk", name="bqk")
            nc.sync.dma_start(bqk_col[:], bqk_d.rearrange("(t p) -> p t", p=128))
            bv_sb = const.tile([1, C], bf16, tag="bv", name="bv")
            nc.sync.dma_start(bv_sb[:], bv_d[None, :])
            bout_col = const.tile([128, NT], f32, tag="bout", name="bout")
            nc.sync.dma_start(bout_col[:], bout_d.rearrange("(t p) -> p t", p=128))

            stats = const.tile([128, 2 * NT], f32, tag="stats", name="stats")
            A_all = const.tile([128, NT], f32, tag="A_all", name="A_all")
            nB_all = const.tile([128, NT], f32, tag="nB_all", name="nB_all")
            # heads 0-7 in rows 0-7, heads 8-15 in rows 32-39 so each
            # half-reciprocal starts at a 32-aligned partition base
            denom = const.tile([40, L], bf16, tag="denom", name="denom")
            recip = const.tile([40, L], bf16, tag="recip", name="recip")

            xnf = [xnf_p.tile([128, L], f32, tag=f"xnf{t}", name=f"xnf{t}") for t in range(NT)]
            ctxt = [ctx_p.tile([128, L], bf16, tag=f"ctx{t}", name=f"ctx{t}") for t in range(NT)]
            # m-chunks 0..7 hold q (heads 2j, 2j+1), chunks 8..15 hold k
            qk = [qkv_p.tile([128, L], bf16, tag=f"qk{h}", name=f"qk{h}") for h in range(H)]
            vt = [qkv_p.tile([128, 16 * 65], bf16, tag=f"vt{t}", name=f"vt{t}") for t in range(NT)]

            # ================= phases A + B =================
            with tc.tile_pool(name="bw", bufs=1) as bw_p, \
                 tc.tile_pool(name="psab", bufs=2, space="PSUM") as ps_ab:

                wqk = []
                wv = []
                for t in range(NT):
                    w = bw_p.tile([128, 2 * C], bf16, tag=f"wqk{t}", name=f"wqk{t}")
                    nc.sync.dma_start(w[:], wqk_d[t * 128:(t + 1) * 128, :])
                    wqk.append(w)
                for t in range(NT):
                    w = bw_p.tile([128, C], bf16, tag=f"wv{t}", name=f"wv{t}")
                    nc.sync.dma_start(w[:], wv_d[t * 128:(t + 1) * 128, :])
                    wv.append(w)

                # ---- phase A: GroupNorm (streamed over 8 channel tiles);
                # each tile's bf16 cast is emitted inline so the QKV matmuls
                # can start as soon as the first tiles are normalized ----
                xnb = []
                with tc.tile_pool(name="xp", bufs=2) as x_p:
                    for t in range(NT):
                        xt = x_p.tile([128, L], f32, tag="x", name="x")
                        nc.sync.dma_start(xt[:], x_d[t * 128:(t + 1) * 128, :])
                        # per-partition sum / sum of squares
                        nc.vector.tensor_reduce(
                            stats[:, 2 * t:2 * t + 1], xt[:], axis=AX.X, op=ALU.add)
                        sqs = x_p.tile([128, L], bf16, tag="sq", name="sq")
                        nc.scalar.activation(
                            sqs[:], xt[:], AF.Square,
                            accum_out=stats[:, 2 * t + 1:2 * t + 2])
                        # cross-partition group sums via block-diagonal matmul
                        gps = ps_ab.tile([128, 2], f32, tag="gn", name="gn")
                        nc.tensor.matmul(
                            gps[:], lhsT=E[:], rhs=stats[:, 2 * t:2 * t + 2],
                            start=True, stop=True)
                        mue = small.tile([128, 2], f32, tag="mue", name="mue")
                        nc.vector.tensor_scalar(
                            out=mue[:], in0=gps[:], scalar1=1.0 / NELEM,
                            scalar2=None, op0=ALU.mult)
                        vneg = small.tile([128, 1], f32, tag="vneg", name="vneg")
                        nc.vector.scalar_tensor_tensor(
                            out=vneg[:], in0=mue[:, 0:1], scalar=mue[:, 0:1],
                            in1=mue[:, 1:2], op0=ALU.mult, op1=ALU.subtract)
                        sd = small.tile([128, 1], f32, tag="sd", name="sd")
                        nc.scalar.activation(
                            sd[:], vneg[:], AF.Sqrt, bias=eps_t[:], scale=-1.0)
                        rsig = small.tile([128, 1], f32, tag="rsig", name="rsig")
                        nc.vector.reciprocal(rsig[:], sd[:])
                        nc.vector.tensor_tensor(
                            out=A_all[:, t:t + 1], in0=rsig[:],
                            in1=gnw_sb[:, t:t + 1], op=ALU.mult)
                        nc.vector.scalar_tensor_tensor(
                            out=nB_all[:, t:t + 1], in0=mue[:, 0:1],
                            scalar=A_all[:, t:t + 1], in1=gnb_sb[:, t:t + 1],
                            op0=ALU.mult, op1=ALU.subtract)
                        # xn = x*A - negB  (fp32, kept for the residual)
                        nc.vector.tensor_scalar(
                            out=xnf[t][:], in0=xt[:],
                            scalar1=A_all[:, t:t + 1], scalar2=nB_all[:, t:t + 1],
                            op0=ALU.mult, op1=ALU.subtract)
                        xb = bw_p.tile([128, L], bf16, tag=f"xnb{t}",
                                       name=f"xnb{t}")
                        nc.scalar.copy(xb[:], xnf[t][:])
                        xnb.append(xb)

                # ---- phase B: QKV matmuls ----
                for h in range(H):
                    ps = ps_ab.tile([128, L], f32, tag="mm", name="mm")
                    for lc in range(2):
                        sl = slice(lc * 512, (lc + 1) * 512)
                        for kc in range(NT):
                            nc.tensor.matmul(
                                ps[:, sl],
                                lhsT=wqk[kc][:, h * 128:(h + 1) * 128],
                                rhs=xnb[kc][:, sl],
                                start=(kc == 0), stop=(kc == NT - 1))
                    # bias fused into the PSUM evacuation (per-partition add)
                    nc.vector.tensor_scalar(
                        out=qk[h][:], in0=ps[:],
                        scalar1=bqk_col[:, h:h + 1], scalar2=None, op0=ALU.add)

                for lt in range(NT):
                    ps = ps_ab.tile([128, L], f32, tag="mm", name="mm")
                    for ncn in range(2):
                        sl = slice(ncn * 512, (ncn + 1) * 512)
                        for kc in range(NT):
                            nc.tensor.matmul(
                                ps[:, sl],
                                lhsT=xnb[kc][:, lt * 128:(lt + 1) * 128],
                                rhs=wv[kc][:, sl],
                                start=(kc == 0), stop=False)
                        nc.tensor.matmul(
                            ps[:, sl],
                            lhsT=ones_bf[0:1, 0:128],
                            rhs=bv_sb[0:1, sl],
                            start=False, stop=True)
                    v3 = vt[lt][:].rearrange("p (h c) -> p h c", c=65)
                    nc.gpsimd.memset(v3[:, :, 64:65], 1.0)
                    nc.scalar.copy(
                        v3[:, :, 0:64],
                        ps[:].rearrange("p (h c) -> p h c", c=64))

            if debug:
                nc.sync.dma_start(dbg["dbg_stats"][:], stats[:])
                for t in range(NT):
                    nc.sync.dma_start(dbg["dbg_xnf"][t], xnf[t][:])
                for h in range(H):
                    nc.sync.dma_start(dbg["dbg_qk"][h], qk[h][:])
                for t in range(NT):
                    nc.sync.dma_start(dbg["dbg_vt"][t], vt[t][:])

            # wout is only needed in phase D; load it after the B-phase
            # weight pool is released to keep peak SBUF under budget
            wout_p = es.enter_context(tc.tile_pool(name="woutp", bufs=1))
            wout = []
            for t in range(NT):
                w = wout_p.tile([128, C], bf16, tag=f"wout{t}", name=f"wout{t}")
                nc.sync.dma_start(w[:], wout_d[t * 128:(t + 1) * 128, :])
                wout.append(w)

            # ================= phase C: attention =================
            with tc.tile_pool(name="expp", bufs=32) as exp_p, \
                 tc.tile_pool(name="psc", bufs=2, space="PSUM") as ps_c:

                def score_pair(j):
                    """Scores for heads 2j (PE rows 0:64) and 2j+1 (rows
                    64:128): adjacent matmuls on disjoint row groups run
                    concurrently in the array."""
                    expsE, expsO = [], []
                    for mc in range(NT):
                        sps = [ps_c.tile([128, L], f32, tag="score",
                                         name="score") for _ in range(2)]
                        for lc in range(2):
                            sl = slice(lc * 512, (lc + 1) * 512)
                            for par in range(2):
                                hp = par * 64
                                nc.tensor.matmul(
                                    sps[par][:, sl],
                                    lhsT=qk[8 + j][hp:hp + 64, mc * 128:(mc + 1) * 128],
                                    rhs=qk[j][hp:hp + 64, sl],
                                    start=True, stop=True)
                        for par, lst in ((0, expsE), (1, expsO)):
                            ex = exp_p.tile([128, L], bf16, tag="expT",
                                            name="expT")
                            nc.scalar.activation(ex[:], sps[par][:], AF.Exp)
                            lst.append(ex)
                    return expsE, expsO

                def ctxmm(h, exps):
                    cps = ps_c.tile([65, L], f32, tag="ctx", name="ctx")
                    for lc in range(2):
                        sl = slice(lc * 512, (lc + 1) * 512)
                        for mc in range(NT):
                            nc.tensor.matmul(
                                cps[:, sl],
                                lhsT=vt[mc][:, h * 65:h * 65 + 65],
                                rhs=exps[mc][:, sl],
                                start=(mc == 0), stop=(mc == NT - 1))
                    # DVE cannot write partition bases not divisible by 32
                    # and DMA cannot read PSUM: stage the denominator row at
                    # partition 0, then DMA it into its denom slot
                    stage = small.tile([1, L], bf16, tag="dstage", name="dstage")
                    nc.vector.tensor_copy(stage[:], cps[64:65, :])
                    dr = h if h < 8 else 24 + h
                    nc.sync.dma_start(denom[dr:dr + 1, :], stage[:])
                    half = (h % 2) * 64
                    nc.vector.tensor_copy(
                        ctxt[h // 2][half:half + 64, :], cps[0:64, :])

                pairs = {0: score_pair(0)}
                if debug:
                    for mc in range(NT):
                        nc.sync.dma_start(dbg["dbg_exp0"][mc],
                                          pairs[0][0][mc][:])
                for j in range(NT):
                    if j + 1 < NT:
                        pairs[j + 1] = score_pair(j + 1)
                    eE, eO = pairs.pop(j)
                    ctxmm(2 * j, eE)
                    ctxmm(2 * j + 1, eO)
                    if j == 4:
                        # first-half reciprocal overlaps remaining attention
                        with nc.allow_low_precision(
                                reason="bf16 softmax denominators (~1024); "
                                       "validated error budget"):
                            nc.vector.reciprocal(recip[0:8, :], denom[0:8, :])

            # ================= phase D: normalize + out-proj =================
            with tc.tile_pool(name="psd", bufs=2, space="PSUM") as ps_d, \
                 tc.tile_pool(name="psd1", bufs=1, space="PSUM") as ps_d1, \
                 tc.tile_pool(name="resp", bufs=3) as res_p, \
                 tc.tile_pool(name="dnorm", bufs=1) as dn_p:

                if debug:
                    for _h in range(H):
                        _dr = _h if _h < 8 else 24 + _h
                        nc.sync.dma_start(dbg["dbg_denom"][_h:_h+1], denom[_dr:_dr+1, :])
                    for t in range(NT):
                        nc.sync.dma_start(dbg["dbg_ctxu"][t], ctxt[t][:])
                with nc.allow_low_precision(
                        reason="bf16 softmax denominators (~1024); "
                               "validated error budget"):
                    nc.vector.reciprocal(recip[32:40, :], denom[32:40, :])
                # DMA each recip row to partition 0/64 of a per-head-pair
                # tile, broadcast across partitions with a K=1 ones-row
                # matmul into PSUM, and fuse the normalize multiply with the
                # PSUM operand on DVE
                recipq = [dn_p.tile([65, L], bf16, tag=f"recipq{q}",
                                    name=f"recipq{q}") for q in range(NT)]
                for h in range(H):
                    rr = h if h < 8 else 24 + h
                    nc.sync.dma_start(
                        recipq[h // 2][64 * (h % 2):64 * (h % 2) + 1, :],
                        recip[rr:rr + 1, :])

                for ct in range(NT):
                    rbps = ps_d1.tile([128, L], f32, tag="rb", name="rb")
                    for half in range(2):
                        hb = 64 * half
                        for lc in range(2):
                            sl = slice(lc * 512, (lc + 1) * 512)
                            nc.tensor.matmul(
                                rbps[hb:hb + 64, sl],
                                lhsT=ones2[hb:hb + 1, :],
                                rhs=recipq[ct][hb:hb + 1, sl],
                                start=True, stop=True)
                    nc.vector.scalar_tensor_tensor(
                        out=ctxt[ct][:], in0=ctxt[ct][:], scalar=1.0,
                        in1=rbps[:], op0=ALU.mult, op1=ALU.mult)

                if debug:
                    for _h in range(H):
                        _dr = _h if _h < 8 else 24 + _h
                        nc.sync.dma_start(dbg["dbg_recip"][_h:_h+1], recip[_dr:_dr+1, :])
                    for t in range(NT):
                        nc.sync.dma_start(dbg["dbg_ctxn"][t], ctxt[t][:])
                for ot in range(NT):
                    ps = ps_d.tile([128, L], f32, tag="out", name="out")
                    for lc in range(2):
                        sl = slice(lc * 512, (lc + 1) * 512)
                        for kc in range(NT):
                            nc.tensor.matmul(
                                ps[:, sl],
                                lhsT=wout[kc][:, ot * 128:(ot + 1) * 128],
                                rhs=ctxt[kc][:, sl],
                                start=(kc == 0), stop=(kc == NT - 1))
                    res = res_p.tile([128, L], f32, tag="res", name="res")
                    # out-proj bias and the GroupNorm residual fused into
                    # one pass: res = (psum + bout) + xn
                    nc.vector.scalar_tensor_tensor(
                        out=res[:], in0=ps[:], scalar=bout_col[:, ot:ot + 1],
                        in1=xnf[ot][:], op0=ALU.add, op1=ALU.add)
                    nc.sync.dma_start(out_d[ot * 128:(ot + 1) * 128, :], res[:])

    nc.compile()
    return nc


def _get_nc(debug=False):
    key = f"nc{int(debug)}"
    if key not in _STATE:
        _STATE[key] = _build_nc(debug)
    return _STATE[key]


def _prep_in_maps(x, gn_weight, gn_bias, qkv_w, qkv_b, out_w, out_b):
    bf16 = ml_dtypes.bfloat16
    x = np.ascontiguousarray(np.asarray(x, np.float32))
    gn_w = np.ascontiguousarray(np.asarray(gn_weight, np.float32))
    gn_b = np.ascontiguousarray(np.asarray(gn_bias, np.float32))
    qkv_w = np.asarray(qkv_w, np.float32)
    qkv_b = np.asarray(qkv_b, np.float32)
    out_w = np.asarray(out_w, np.float32)
    out_b = np.asarray(out_b, np.float32)

    wr = qkv_w.reshape(H, 3, D, C)
    br = qkv_b.reshape(H, 3, D)
    # layout: rows 0..1023 = q (pre-scaled by 1/64 = scale^2 folded into one
    # side; exact power of 2), rows 1024..2047 = k; both ordered h*64+d
    wqk = np.concatenate(
        [(wr[:, 0] / 64.0).reshape(C, C), wr[:, 1].reshape(C, C)], axis=0)
    bqk = np.concatenate(
        [(br[:, 0] / 64.0).reshape(C), br[:, 1].reshape(C)], axis=0)
    wqk_T = np.ascontiguousarray(wqk.T).astype(bf16)
    bqk = bqk.astype(np.float32)
    wv_T = np.ascontiguousarray(wr[:, 2].reshape(C, C).T).astype(bf16)
    bv = br[:, 2].reshape(C).astype(bf16)
    wout_T = np.ascontiguousarray(out_w.T).astype(bf16)
    bout = out_b.astype(np.float32)

    shared = {
        "gnw": gn_w, "gnb": gn_b,
        "wqk": wqk_T, "bqk": bqk,
        "wv": wv_T, "bv": bv,
        "wout": wout_T, "bout": bout,
    }
    return [dict(shared, x=np.ascontiguousarray(x[b])) for b in range(B)]


def _run(in_maps, trace=False, trace_cores=None, debug=False):
    from concourse.bass_utils import run_bass_kernel_spmd
    nc = _get_nc(debug)
    kwargs = {}
    if trace:
        kwargs.update(trace=True)
        if trace_cores is not None:
            kwargs.update(trace_cores=trace_cores)
    return run_bass_kernel_spmd(nc, in_maps, list(range(B)), **kwargs)


def kernel(x, gn_weight, gn_bias, qkv_w, qkv_b, out_w, out_b, qk_bias=0):
    # qk_bias is a scalar added to every attention score; softmax over the
    # last axis is invariant to it, so it does not affect the output.
    in_maps = _prep_in_maps(x, gn_weight, gn_bias, qkv_w, qkv_b, out_w, out_b)
    res = _run(in_maps)
    out = np.stack([np.asarray(res.results[b]["out"]) for b in range(B)])
    return out.astype(np.float32)


# revision 26
# speedup vs baseline: 1.1793x; 1.1793x over previous
"""Trainium2 Bass kernel for nn_AttentionBlock (GroupNorm32 + 16-head self
attention + out-proj + residual), data-parallel over batch across 8 cores.

Contract: kernel(**inputs) takes the FULL unsharded inputs (as produced by
reference.setup_inputs) and returns the FULL [8, 1024, 1024] fp32 output.

Per-core algorithm (one batch element per NeuronCore, no collectives):
  A. GroupNorm in fp32. Per 128-channel tile: per-partition sum (DVE reduce)
     and sum-of-squares (ACT Square with accum_out), group reduction via a
     tiny block-diagonal matmul on PE, then xn = x*A - negB per partition.
  B. QKV as two matmuls in bf16. q/k produced channel-major (q rows pre-scaled
     by 1/64 on the host, biases folded in via K=1 ones-row matmuls);
     v produced transposed (positions-major) so attention needs no transposes.
  C. Per head: scoreT = k.T @ q (keys on PSUM partitions), exp on ACT into
     bf16 expT, then ctxT = [vT | ones].T @ expT -- the appended ones column
     yields the softmax denominator row for free.
  D. Normalize ctx by broadcast reciprocal denominators, out-proj matmul in
     bf16 (bias folded), fp32 residual with the GroupNorm output, DMA out.
"""

import sys
import numpy as np

if "/opt/trn_rl_repo" not in sys.path:
    sys.path.insert(0, "/opt/trn_rl_repo")

import ml_dtypes

B, C, L = 8, 1024, 1024
H, D = 16, 64
NT = 8            # 128-row tiles per 1024 dim
EPS = 1e-5
NELEM = 32 * L    # elements per GroupNorm group (32 channels x 1024)

_STATE = {}


def _build_nc(debug=False):
    import concourse.bacc as bacc
    import concourse.mybir as mybir
    import concourse.tile as tile

    f32 = mybir.dt.float32
    bf16 = mybir.dt.bfloat16
    AX = mybir.AxisListType
    ALU = mybir.AluOpType
    AF = mybir.ActivationFunctionType

    nc = bacc.Bacc("TRN2", target_bir_lowering=False, debug=False, num_devices=8)

    x_d = nc.dram_tensor("x", [C, L], f32, kind="ExternalInput").ap()
    gnw_d = nc.dram_tensor("gnw", [C], f32, kind="ExternalInput").ap()
    gnb_d = nc.dram_tensor("gnb", [C], f32, kind="ExternalInput").ap()
    wqk_d = nc.dram_tensor("wqk", [C, 2 * C], bf16, kind="ExternalInput").ap()
    bqk_d = nc.dram_tensor("bqk", [2 * C], f32, kind="ExternalInput").ap()
    wv_d = nc.dram_tensor("wv", [C, C], bf16, kind="ExternalInput").ap()
    bv_d = nc.dram_tensor("bv", [C], bf16, kind="ExternalInput").ap()
    wout_d = nc.dram_tensor("wout", [C, C], bf16, kind="ExternalInput").ap()
    bout_d = nc.dram_tensor("bout", [C], f32, kind="ExternalInput").ap()
    out_d = nc.dram_tensor("out", [C, L], f32, kind="ExternalOutput").ap()
    dbg = {}
    if debug:
        for nm, shp, dt in [
            ("dbg_stats", [128, 16], f32), ("dbg_xnf", [NT, 128, L], f32),
            ("dbg_qk", [H, 128, L], bf16), ("dbg_vt", [NT, 128, 16 * 65], bf16),
            ("dbg_exp0", [NT, 128, L], bf16), ("dbg_denom", [16, L], bf16),
            ("dbg_recip", [16, L], bf16), ("dbg_ctxu", [NT, 128, L], bf16),
            ("dbg_ctxn", [NT, 128, L], bf16),
        ]:
            dbg[nm] = nc.dram_tensor(nm, shp, dt, kind="ExternalOutput").ap()

    with tile.TileContext(nc) as tc:
        # ---------------- persistent pools ----------------
        from contextlib import ExitStack

        with ExitStack() as es:
            const = es.enter_context(tc.tile_pool(name="const", bufs=1))
            small = es.enter_context(tc.tile_pool(name="small", bufs=3))
            xnf_p = es.enter_context(tc.tile_pool(name="xnf", bufs=1))
            ctx_p = es.enter_context(tc.tile_pool(name="ctxp", bufs=1))
            qkv_p = es.enter_context(tc.tile_pool(name="qkvp", bufs=1))

            # constants and small parameters
            E = const.tile([128, 128], f32, tag="E", name="E")
            nc.gpsimd.memset(E[:], 0.0)
            for j in range(4):
                nc.gpsimd.memset(E[32 * j:32 * (j + 1), 32 * j:32 * (j + 1)], 1.0)
            ones_bf = const.tile([1, 512], bf16, tag="ones", name="ones")
            nc.gpsimd.memset(ones_bf[:], 1.0)
            # ones rows at partition 0 and 64 (broadcast matmuls need the
            # stationary ones row at the same base partition as the source)
            ones2 = const.tile([65, 64], bf16, tag="ones2", name="ones2")
            nc.gpsimd.memset(ones2[0:1, :], 1.0)
            nc.gpsimd.memset(ones2[64:65, :], 1.0)
            eps_t = const.tile([128, 1], f32, tag="eps", name="eps")
            nc.gpsimd.memset(eps_t[:], EPS)

            gnw_sb = const.tile([128, NT], f32, tag="gnw", name="gnw")
            nc.sync.dma_start(gnw_sb[:], gnw_d.rearrange("(t p) -> p t", p=128))
            gnb_sb = const.tile([128, NT], f32, tag="gnb", name="gnb")
            nc.sync.dma_start(gnb_sb[:], gnb_d.rearrange("(t p) -> p t", p=128))
            bqk_col = const.tile([128, H], f32, tag="bqk", name="bqk")
            nc.sync.dma_start(bqk_col[:], bqk_d.rearrange("(t p) -> p t", p=128))
            bv_sb = const.tile([1, C], bf16, tag="bv", name="bv")
            nc.sync.dma_start(bv_sb[:], bv_d[None, :])
            bout_col = const.tile([128, NT], f32, tag="bout", name="bout")
            nc.sync.dma_start(bout_col[:], bout_d.rearrange("(t p) -> p t", p=128))

            stats = const.tile([128, 2 * NT], f32, tag="stats", name="stats")
            A_all = const.tile([128, NT], f32, tag="A_all", name="A_all")
            nB_all = const.tile([128, NT], f32, tag="nB_all", name="nB_all")
            # heads 0-7 in rows 0-7, heads 8-15 in rows 32-39 so each
            # half-reciprocal starts at a 32-aligned partition base
            denom = const.tile([40, L], bf16, tag="denom", name="denom")
            recip = const.tile([40, L], bf16, tag="recip", name="recip")

            xnf = [xnf_p.tile([128, L], f32, tag=f"xnf{t}", name=f"xnf{t}") for t in range(NT)]
            ctxt = [ctx_p.tile([128, L], bf16, tag=f"ctx{t}", name=f"ctx{t}") for t in range(NT)]
            # m-chunks 0..7 hold q (heads 2j, 2j+1), chunks 8..15 hold k
            qk = [qkv_p.tile([128, L], bf16, tag=f"qk{h}", name=f"qk{h}") for h in range(H)]
            vt = [qkv_p.tile([128, 16 * 65], bf16, tag=f"vt{t}", name=f"vt{t}") for t in range(NT)]

            # ================= phases A + B =================
            with tc.tile_pool(name="bw", bufs=1) as bw_p, \
                 tc.tile_pool(name="psab", bufs=2, space="PSUM") as ps_ab:

                wqk = []
                wv = []
                for t in range(NT):
                    w = bw_p.tile([128, 2 * C], bf16, tag=f"wqk{t}", name=f"wqk{t}")
                    nc.sync.dma_start(w[:], wqk_d[t * 128:(t + 1) * 128, :])
                    wqk.append(w)
                for t in range(NT):
                    w = bw_p.tile([128, C], bf16, tag=f"wv{t}", name=f"wv{t}")
                    nc.sync.dma_start(w[:], wv_d[t * 128:(t + 1) * 128, :])
                    wv.append(w)

                # ---- phase A: GroupNorm (streamed over 8 channel tiles) ----
                with tc.tile_pool(name="xp", bufs=2) as x_p:
                    for t in range(NT):
                        xt = x_p.tile([128, L], f32, tag="x", name="x")
                        nc.sync.dma_start(xt[:], x_d[t * 128:(t + 1) * 128, :])
                        # per-partition sum / sum of squares
                        nc.vector.tensor_reduce(
                            stats[:, 2 * t:2 * t + 1], xt[:], axis=AX.X, op=ALU.add)
                        sqs = x_p.tile([128, L], bf16, tag="sq", name="sq")
                        nc.scalar.activation(
                            sqs[:], xt[:], AF.Square,
                            accum_out=stats[:, 2 * t + 1:2 * t + 2])
                        # cross-partition group sums via block-diagonal matmul
                        gps = ps_ab.tile([128, 2], f32, tag="gn", name="gn")
                        nc.tensor.matmul(
                            gps[:], lhsT=E[:], rhs=stats[:, 2 * t:2 * t + 2],
                            start=True, stop=True)
                        mue = small.tile([128, 2], f32, tag="mue", name="mue")
                        nc.vector.tensor_scalar(
                            out=mue[:], in0=gps[:], scalar1=1.0 / NELEM,
                            scalar2=None, op0=ALU.mult)
                        vneg = small.tile([128, 1], f32, tag="vneg", name="vneg")
                        nc.vector.scalar_tensor_tensor(
                            out=vneg[:], in0=mue[:, 0:1], scalar=mue[:, 0:1],
                            in1=mue[:, 1:2], op0=ALU.mult, op1=ALU.subtract)
                        sd = small.tile([128, 1], f32, tag="sd", name="sd")
                        nc.scalar.activation(
                            sd[:], vneg[:], AF.Sqrt, bias=eps_t[:], scale=-1.0)
                        rsig = small.tile([128, 1], f32, tag="rsig", name="rsig")
                        nc.vector.reciprocal(rsig[:], sd[:])
                        nc.vector.tensor_tensor(
                            out=A_all[:, t:t + 1], in0=rsig[:],
                            in1=gnw_sb[:, t:t + 1], op=ALU.mult)
                        nc.vector.scalar_tensor_tensor(
                            out=nB_all[:, t:t + 1], in0=mue[:, 0:1],
                            scalar=A_all[:, t:t + 1], in1=gnb_sb[:, t:t + 1],
                            op0=ALU.mult, op1=ALU.subtract)
                        # xn = x*A - negB  (fp32, kept for the residual)
                        nc.vector.tensor_scalar(
                            out=xnf[t][:], in0=xt[:],
                            scalar1=A_all[:, t:t + 1], scalar2=nB_all[:, t:t + 1],
                            op0=ALU.mult, op1=ALU.subtract)

                # bf16 copy of xn for the matmuls
                xnb = []
                for t in range(NT):
                    xb = bw_p.tile([128, L], bf16, tag=f"xnb{t}", name=f"xnb{t}")
                    nc.scalar.copy(xb[:], xnf[t][:])
                    xnb.append(xb)

                # ---- phase B: QKV matmuls ----
                for h in range(H):
                    ps = ps_ab.tile([128, L], f32, tag="mm", name="mm")
                    for lc in range(2):
                        sl = slice(lc * 512, (lc + 1) * 512)
                        for kc in range(NT):
                            nc.tensor.matmul(
                                ps[:, sl],
                                lhsT=wqk[kc][:, h * 128:(h + 1) * 128],
                                rhs=xnb[kc][:, sl],
                                start=(kc == 0), stop=(kc == NT - 1))
                    # bias fused into the PSUM evacuation (per-partition add)
                    nc.vector.tensor_scalar(
                        out=qk[h][:], in0=ps[:],
                        scalar1=bqk_col[:, h:h + 1], scalar2=None, op0=ALU.add)

                for lt in range(NT):
                    ps = ps_ab.tile([128, L], f32, tag="mm", name="mm")
                    for ncn in range(2):
                        sl = slice(ncn * 512, (ncn + 1) * 512)
                        for kc in range(NT):
                            nc.tensor.matmul(
                                ps[:, sl],
                                lhsT=xnb[kc][:, lt * 128:(lt + 1) * 128],
                                rhs=wv[kc][:, sl],
                                start=(kc == 0), stop=False)
                        nc.tensor.matmul(
                            ps[:, sl],
                            lhsT=ones_bf[0:1, 0:128],
                            rhs=bv_sb[0:1, sl],
                            start=False, stop=True)
                    v3 = vt[lt][:].rearrange("p (h c) -> p h c", c=65)
                    nc.gpsimd.memset(v3[:, :, 64:65], 1.0)
                    nc.scalar.copy(
                        v3[:, :, 0:64],
                        ps[:].rearrange("p (h c) -> p h c", c=64))

            if debug:
                nc.sync.dma_start(dbg["dbg_stats"][:], stats[:])
                for t in range(NT):
                    nc.sync.dma_start(dbg["dbg_xnf"][t], xnf[t][:])
                for h in range(H):
                    nc.sync.dma_start(dbg["dbg_qk"][h], qk[h][:])
                for t in range(NT):
                    nc.sync.dma_start(dbg["dbg_vt"][t], vt[t][:])

            # wout is only needed in phase D; load it after the B-phase
            # weight pool is released to keep peak SBUF under budget
            wout_p = es.enter_context(tc.tile_pool(name="woutp", bufs=1))
            wout = []
            for t in range(NT):
                w = wout_p.tile([128, C], bf16, tag=f"wout{t}", name=f"wout{t}")
                nc.sync.dma_start(w[:], wout_d[t * 128:(t + 1) * 128, :])
                wout.append(w)

            # ================= phase C: attention =================
            with tc.tile_pool(name="expp", bufs=32) as exp_p, \
                 tc.tile_pool(name="psc", bufs=2, space="PSUM") as ps_c:

                def score_pair(j):
                    """Scores for heads 2j (PE rows 0:64) and 2j+1 (rows
                    64:128): adjacent matmuls on disjoint row groups run
                    concurrently in the array."""
                    expsE, expsO = [], []
                    for mc in range(NT):
                        sps = [ps_c.tile([128, L], f32, tag="score",
                                         name="score") for _ in range(2)]
                        for lc in range(2):
                            sl = slice(lc * 512, (lc + 1) * 512)
                            for par in range(2):
                                hp = par * 64
                                nc.tensor.matmul(
                                    sps[par][:, sl],
                                    lhsT=qk[8 + j][hp:hp + 64, mc * 128:(mc + 1) * 128],
                                    rhs=qk[j][hp:hp + 64, sl],
                                    start=True, stop=True)
                        for par, lst in ((0, expsE), (1, expsO)):
                            ex = exp_p.tile([128, L], bf16, tag="expT",
                                            name="expT")
                            nc.scalar.activation(ex[:], sps[par][:], AF.Exp)
                            lst.append(ex)
                    return expsE, expsO

                def ctxmm(h, exps):
                    cps = ps_c.tile([65, L], f32, tag="ctx", name="ctx")
                    for lc in range(2):
                        sl = slice(lc * 512, (lc + 1) * 512)
                        for mc in range(NT):
                            nc.tensor.matmul(
                                cps[:, sl],
                                lhsT=vt[mc][:, h * 65:h * 65 + 65],
                                rhs=exps[mc][:, sl],
                                start=(mc == 0), stop=(mc == NT - 1))
                    # DVE cannot write partition bases not divisible by 32
                    # and DMA cannot read PSUM: stage the denominator row at
                    # partition 0, then DMA it into its denom slot
                    stage = small.tile([1, L], bf16, tag="dstage", name="dstage")
                    nc.vector.tensor_copy(stage[:], cps[64:65, :])
                    dr = h if h < 8 else 24 + h
                    nc.sync.dma_start(denom[dr:dr + 1, :], stage[:])
                    half = (h % 2) * 64
                    nc.vector.tensor_copy(
                        ctxt[h // 2][half:half + 64, :], cps[0:64, :])

                pairs = {0: score_pair(0)}
                if debug:
                    for mc in range(NT):
                        nc.sync.dma_start(dbg["dbg_exp0"][mc],
                                          pairs[0][0][mc][:])
                for j in range(NT):
                    if j + 1 < NT:
                        pairs[j + 1] = score_pair(j + 1)
                    eE, eO = pairs.pop(j)
                    ctxmm(2 * j, eE)
                    ctxmm(2 * j + 1, eO)
                    if j == 4:
                        # first-half reciprocal overlaps remaining attention
                        with nc.allow_low_precision(
                                reason="bf16 softmax denominators (~1024); "
                                       "validated error budget"):
                            nc.vector.reciprocal(recip[0:8, :], denom[0:8, :])

            # ================= phase D: normalize + out-proj =================
            with tc.tile_pool(name="psd", bufs=2, space="PSUM") as ps_d, \
                 tc.tile_pool(name="psd1", bufs=1, space="PSUM") as ps_d1, \
                 tc.tile_pool(name="resp", bufs=3) as res_p, \
                 tc.tile_pool(name="dnorm", bufs=1) as dn_p:

                if debug:
                    for _h in range(H):
                        _dr = _h if _h < 8 else 24 + _h
                        nc.sync.dma_start(dbg["dbg_denom"][_h:_h+1], denom[_dr:_dr+1, :])
                    for t in range(NT):
                        nc.sync.dma_start(dbg["dbg_ctxu"][t], ctxt[t][:])
                with nc.allow_low_precision(
                        reason="bf16 softmax denominators (~1024); "
                               "validated error budget"):
                    nc.vector.reciprocal(recip[32:40, :], denom[32:40, :])
                # DMA each recip row to partition 0/64 of a per-head-pair
                # tile, broadcast across partitions with a K=1 ones-row
                # matmul into PSUM, and fuse the normalize multiply with the
                # PSUM operand on DVE
                recipq = [dn_p.tile([65, L], bf16, tag=f"recipq{q}",
                                    name=f"recipq{q}") for q in range(NT)]
                for h in range(H):
                    rr = h if h < 8 else 24 + h
                    nc.sync.dma_start(
                        recipq[h // 2][64 * (h % 2):64 * (h % 2) + 1, :],
                        recip[rr:rr + 1, :])

                for ct in range(NT):
                    rbps = ps_d1.tile([128, L], f32, tag="rb", name="rb")
                    for half in range(2):
                        hb = 64 * half
                        for lc in range(2):
                            sl = slice(lc * 512, (lc + 1) * 512)
                            nc.tensor.matmul(
                                rbps[hb:hb + 64, sl],
                                lhsT=ones2[hb:hb + 1, :],
                                rhs=recipq[ct][hb:hb + 1, sl],
                                start=True, stop=True)
                    nc.vector.scalar_tensor_tensor(
                        out=ctxt[ct][:], in0=ctxt[ct][:], scalar=1.0,
                        in1=rbps[:], op0=ALU.mult, op1=ALU.mult)

                if debug:
                    for _h in range(H):
                        _dr = _h if _h < 8 else 24 + _h
                        nc.sync.dma_start(dbg["dbg_recip"][_h:_h+1], recip[_dr:_dr+1, :])
                    for t in range(NT):
                        nc.sync.dma_start(dbg["dbg_ctxn"][t], ctxt[t][:])
                for ot in range(NT):
                    ps = ps_d.tile([128, L], f32, tag="out", name="out")
                    for lc in range(2):
                        sl = slice(lc * 512, (lc + 1) * 512)
                        for kc in range(NT):
                            nc.tensor.matmul(
                                ps[:, sl],
                                lhsT=wout[kc][:, ot * 128:(ot + 1) * 128],
                                rhs=ctxt[kc][:, sl],
                                start=(kc == 0), stop=(kc == NT - 1))
                    res = res_p.tile([128, L], f32, tag="res", name="res")
                    # out-proj bias and the GroupNorm residual fused into
                    # one pass: res = (psum + bout) + xn
                    nc.vector.scalar_tensor_tensor(
                        out=res[:], in0=ps[:], scalar=bout_col[:, ot:ot + 1],
                        in1=xnf[ot][:], op0=ALU.add, op1=ALU.add)
                    nc.sync.dma_start(out_d[ot * 128:(ot + 1) * 128, :], res[:])

    nc.compile()
    return nc


def _get_nc(debug=False):
    key = f"nc{int(debug)}"
    if key not in _STATE:
        _STATE[key] = _build_nc(debug)
    return _STATE[key]


def _prep_in_maps(x, gn_weight, gn_bias, qkv_w, qkv_b, out_w, out_b):
    bf16 = ml_dtypes.bfloat16
    x = np.ascontiguousarray(np.asarray(x, np.float32))
    gn_w = np.ascontiguousarray(np.asarray(gn_weight, np.float32))
    gn_b = np.ascontiguousarray(np.asarray(gn_bias, np.float32))
    qkv_w = np.asarray(qkv_w, np.float32)
    qkv_b = np.asarray(qkv_b, np.float32)
    out_w = np.asarray(out_w, np.float32)
    out_b = np.asarray(out_b, np.float32)

    wr = qkv_w.reshape(H, 3, D, C)
    br = qkv_b.reshape(H, 3, D)
    # layout: rows 0..1023 = q (pre-scaled by 1/64 = scale^2 folded into one
    # side; exact power of 2), rows 1024..2047 = k; both ordered h*64+d
    wqk = np.concatenate(
        [(wr[:, 0] / 64.0).reshape(C, C), wr[:, 1].reshape(C, C)], axis=0)
    bqk = np.concatenate(
        [(br[:, 0] / 64.0).reshape(C), br[:, 1].reshape(C)], axis=0)
    wqk_T = np.ascontiguousarray(wqk.T).astype(bf16)
    bqk = bqk.astype(np.float32)
    wv_T = np.ascontiguousarray(wr[:, 2].reshape(C, C).T).astype(bf16)
    bv = br[:, 2].reshape(C).astype(bf16)
    wout_T = np.ascontiguousarray(out_w.T).astype(bf16)
    bout = out_b.astype(np.float32)

    shared = {
        "gnw": gn_w, "gnb": gn_b,
        "wqk": wqk_T, "bqk": bqk,
        "wv": wv_T, "bv": bv,
        "wout": wout_T, "bout": bout,
    }
    return [dict(shared, x=np.ascontiguousarray(x[b])) for b in range(B)]


def _run(in_maps, trace=False, trace_cores=None, debug=False):
    from concourse.bass_utils import run_bass_kernel_spmd
    nc = _get_nc(debug)
    kwargs = {}
    if trace:
        kwargs.update(trace=True)
        if trace_cores is not None:
            kwargs.update(trace_cores=trace_cores)
    return run_bass_kernel_spmd(nc, in_maps, list(range(B)), **kwargs)


def kernel(x, gn_weight, gn_bias, qkv_w, qkv_b, out_w, out_b, qk_bias=0):
    # qk_bias is a scalar added to every attention score; softmax over the
    # last axis is invariant to it, so it does not affect the output.
    in_maps = _prep_in_maps(x, gn_weight, gn_bias, qkv_w, qkv_b, out_w, out_b)
    res = _run(in_maps)
    out = np.stack([np.asarray(res.results[b]["out"]) for b in range(B)])
    return out.astype(np.float32)
